# revision 104
# speedup vs baseline: 1.1614x; 1.0000x over previous
"""Trainium2 Bass kernel: 3-layer GAT + BN + ELU + residual + global mean pool + linear.

Sharding: nodes (and their incident edges, grouped by destination) are
sharded across 8 NeuronCores. Weights replicated.

Key structure (cost-model driven; the scheduler bills a dma_gather as
output-free-size elements x Pool cycle, exclusively on Pool, and a
collective as ~15us flat on its issuing engine — Pool is the only
engine the compiler accepts for collectives):
  - Per-head basis embedding: within each head's 64-dim block of
    h = a@W, change basis to R_h = [Q_h(62) | ad_h | as_h] (Q_h an
    orthonormal complement). The table row t = h@R then carries the
    attention logits sD_h, sS_h in dims 62/63 of each head block, so
    the gather row is exactly 512 elements (1024B, %256) instead of
    640. After aggregation y = (U/Z) @ blockdiag(R_h^-1) recovers the
    standard basis (division by the per-head Z commutes with the
    within-head basis change).
  - Nodes are relabeled per core (greedy LPT on degree) so every
    dst-block has ~equal edge count -> uniform 16-slot blocks, exactly
    two 8-slot gather chunks each (ramp-in/out blocks use finer
    chunk schedules to shorten the DVE drain at the phase tail).
  - Per-block tails (U/Z division, basis recovery, BN stat matmuls)
    are software-pipelined: emitted after the NEXT block's head so the
    in-order DVE/PE queues interleave two blocks.
  - Self-loop prep (logits/exp/weighted rows) for all blocks is
    hoisted before each edge phase to run under the collective.
  - ilv layout [c, h] (h fastest) for the alpha multiply (DVE 2x);
    sD at ilv 496:504, sS at 504:512, both contiguous. L3 (heads=1)
    duplicates w into pairs so its alpha multiply also hits DVE 2x.
  - BN stats accumulate transposed ([128,4] col sums via 1-col
    matmuls with a 1/N-valued column, pre-dividing by N); stats
    AllGather is [128,8] f32. rsqrt via Ln+Exp(scale=-0.5) (one
    act-table set). elu via max(v,0)+min(exp(v)-1,0) — all 4x DVE
    ops. scf/shf broadcast rows via rank-1 PE matmul, not Pool.
    Biases b1/b2/b3 dropped (BN shift-invariance); enc_b kept. BN3's
    affine is folded into the final linear (scale linW rows by scf3,
    shift into the bias).
"""
import sys
if '/opt/trn_rl_repo' not in sys.path:
    sys.path.insert(0, '/opt/trn_rl_repo')
import numpy as np
import ml_dtypes

import concourse.bass as bass
import concourse.bacc as bacc
import concourse.mybir as mybir
from concourse import tile
from concourse.bass_utils import run_bass_kernel_spmd

F32 = mybir.dt.float32
FP8 = mybir.dt.float8e4
BF16 = mybir.dt.bfloat16
I16 = mybir.dt.int16
AL = mybir.AluOpType
ACTF = mybir.ActivationFunctionType
AX = mybir.AxisListType

N, E, FIN, H, C, G, NCLS = 10000, 160000, 512, 8, 64, 64, 64
P = 8
NL = N // P            # 1250 nodes per core
NT = 10                # node tiles per core (9x128 + 98)
LAST = NL - 9 * 128    # 98
ROW12 = 512            # bf16 table row (1024B, %256): h@R with sD/sS embedded
ROW3 = 128             # bf16 table row L3 (256B); data in 0:66
SS3 = 66               # h3(64) | sS(1) | sD(1)
EPS_Z = 1e-16
EPS_BN = 1e-5
NP_BF16 = ml_dtypes.bfloat16
CH = 8

# interleave permutation: ilv position c*8+h  <- std position h*64+c
PERM = np.arange(FIN).reshape(H, C).T.reshape(-1)


def _blockdiag(a):
    # a [H, C] -> [H*C, H] with column h holding a[h] in rows h*C:(h+1)*C
    hh, cc = a.shape
    out = np.zeros((hh * cc, hh), np.float64)
    for h in range(hh):
        out[h * cc:(h + 1) * cc, h] = a[h]
    return out


def _headbasis(a_s, a_d):
    """R = blockdiag_h [Q_h(62) | ad_h | as_h], Rinv = R^-1. std basis."""
    Rb = np.zeros((FIN, FIN), np.float64)
    for h in range(H):
        ad = np.asarray(a_d[h], np.float64)
        asv = np.asarray(a_s[h], np.float64)
        M = np.stack([ad, asv], axis=1)                   # [64, 2]
        U, s, _ = np.linalg.svd(M, full_matrices=True)
        assert s[-1] > 1e-6, "attention projections nearly collinear"
        Q = U[:, 2:]                                      # [64, 62] orthonormal
        Rh = np.concatenate([Q, M], axis=1)               # [64, 64]
        Rb[h * C:(h + 1) * C, h * C:(h + 1) * C] = Rh
    return Rb, np.linalg.inv(Rb)


def _balance_blocks(rem_deg, loc_deg, caps, loc_cap=256):
    """Greedy: assign nodes to blocks balancing remote degree while keeping
    each block's local degree under loc_cap (the 2-slot local window).
    Returns newpos[old_local] = new local id."""
    nb = len(caps)
    order = np.argsort(-rem_deg, kind='stable')
    rload = [0.0] * nb
    lload = [0.0] * nb
    room = list(caps)
    members = [[] for _ in range(nb)]
    for nd in order:
        avail = [bb for bb in range(nb) if room[bb] > 0]
        b = min(avail, key=lambda bb: (rload[bb], lload[bb]))
        members[b].append(nd)
        rload[b] += rem_deg[nd]
        lload[b] += loc_deg[nd]
        room[b] -= 1
    newpos = np.zeros(len(rem_deg), np.int64)
    base = 0
    for b in range(nb):
        mem = np.sort(np.asarray(members[b], np.int64))
        newpos[mem] = base + np.arange(len(mem))
        base += caps[b]
    return newpos


def _prep(inputs):
    x = np.asarray(inputs['x'], np.float32)
    ei = np.asarray(inputs['edge_index'], np.int64)
    batch = np.asarray(inputs['batch'], np.int64)

    src = ei[0].astype(np.int64)
    dst = ei[1].astype(np.int64)
    caps = [128] * 9 + [LAST]

    # --- per-core node relabeling: balance remote-degree across blocks ---
    dcore = dst // NL
    scorev = src // NL
    newpos_all = np.zeros(N, np.int64)
    for c in range(P):
        m = dcore == c
        dl_old = dst[m] - c * NL
        remote = (scorev[m] != c)
        deg = np.bincount(dl_old, minlength=NL).astype(np.float64)
        newpos = _balance_blocks(deg, np.zeros(NL), caps)
        newpos_all[c * NL:(c + 1) * NL] = c * NL + newpos
    src_n = newpos_all[src]
    dst_n = newpos_all[dst]
    # old position of each new id (for x / pool relabeling)
    oldpos_all = np.zeros(N, np.int64)
    oldpos_all[newpos_all] = np.arange(N)

    order = np.argsort(dst_n, kind='stable')
    src_n, dst_n = src_n[order], dst_n[order]

    core = dst_n // NL
    blk = (dst_n % NL) // 128
    dloc = (dst_n % NL) % 128
    scr = src_n // NL

    per_cb = {}
    T = np.ones(NT, np.int64)
    for c in range(P):
        m = core == c
        sc, dc, bc = src_n[m], dloc[m], blk[m]
        for b in range(NT):
            mb = bc == b
            per_cb[(c, b)] = (sc[mb], dc[mb])
            T[b] = max(T[b], (int(mb.sum()) + 127) // 128)
    sbase = np.zeros(NT, np.int64)
    sbase[1:] = np.cumsum(T)[:-1]
    TT = int(T.sum())
    NE = TT * 128

    per_core = []
    for c in range(P):
        sidx = np.zeros(NE, np.int64)
        dl = np.full(NE, 255, np.int64)
        for b in range(NT):
            es, ed = per_cb[(c, b)]
            off = int(sbase[b]) * 128
            sidx[off:off + len(es)] = es
            dl[off:off + len(ed)] = ed
        j = np.arange(NE)
        t, pp = j // 128, j % 128
        valid = dl < 128
        S = np.zeros((TT, 128, 128), ml_dtypes.float8_e4m3)
        S[t[valid], pp[valid], dl[valid]] = 1
        S_flat = np.ascontiguousarray(S.transpose(1, 0, 2).reshape(128, TT * 128))
        ST_flat = np.ascontiguousarray(S.transpose(2, 0, 1).reshape(128, TT * 128))
        g16 = np.zeros((16, NE // 16), np.int16)
        g16[j % 16, j // 16] = sidx.astype(np.int16)
        gidx = np.tile(g16, (8, 1))

        xc = x[oldpos_all[c * NL:(c + 1) * NL]]          # [1250, 512] new order
        x0T = np.zeros((FIN, NT * 128), np.float32)
        x0T[:, :NL] = xc.T
        x0T = x0T.astype(NP_BF16)

        cnt = np.bincount(batch, minlength=G).astype(np.float64)
        inv = 1.0 / np.maximum(cnt, 1.0)
        pool = np.zeros((NT, 128, G), np.float32)
        bats = batch[oldpos_all[c * NL:(c + 1) * NL]]
        nn, ppp = np.arange(NL) // 128, np.arange(NL) % 128
        pool[nn, ppp, bats] = inv[bats]

        per_core.append(dict(S=S_flat, ST=ST_flat, gidx=gidx, x0T=x0T,
                             pool=pool))

    f64 = lambda k: np.asarray(inputs[k], np.float64)
    W1, W2, W3 = f64('W1'), f64('W2'), f64('W3')
    encW = f64('enc_W')
    encb = f64('enc_b')
    R1, R1i = _headbasis(f64('as1'), f64('ad1'))
    R2, R2i = _headbasis(f64('as2'), f64('ad2'))

    # L1 table: t1 = x_enc @ W1 @ R1 (512 cols, tab-ilv out)
    RHS_t1 = (encW @ W1 @ R1)[:, PERM]
    eb_t1 = (encb @ W1 @ R1)[PERM][None, :]
    # x_enc plain (residual base), ilv
    RHS_xe = encW[:, PERM]
    eb_xe = encb[PERM][None, :]
    # recover y (std-ilv) from aggregated table: rows tab-ilv, cols std-ilv
    RINV1 = R1i[PERM][:, PERM]
    RINV2 = R2i[PERM][:, PERM]
    # boundary table matmuls: consume a-ilv, produce tab-ilv
    Wtab2 = (W2 @ R2)[PERM][:, PERM]
    # L3 keeps plain form: [h3 | sS3 | sD3] from a2-ilv
    Wc3 = np.concatenate(
        [W3, (W3 @ f64('as3')[0])[:, None], (W3 @ f64('ad3')[0])[:, None]],
        axis=1)[PERM]

    shared = dict(
        rhs_t1=RHS_t1.astype(NP_BF16),
        rhs_xe=RHS_xe.astype(NP_BF16),
        rinv1=RINV1.astype(NP_BF16),
        rinv2=RINV2.astype(NP_BF16),
        wtab2=Wtab2.astype(NP_BF16),
        w3=Wc3.astype(NP_BF16),
        eb_t1=eb_t1.astype(NP_BF16),
        eb_xe=eb_xe.astype(NP_BF16),
        g1T=np.ascontiguousarray(
            np.asarray(inputs['g1'], np.float32)[PERM].reshape(4, 128).T),
        be1T=np.ascontiguousarray(
            np.asarray(inputs['be1'], np.float32)[PERM].reshape(4, 128).T),
        g2T=np.ascontiguousarray(
            np.asarray(inputs['g2'], np.float32)[PERM].reshape(4, 128).T),
        be2T=np.ascontiguousarray(
            np.asarray(inputs['be2'], np.float32)[PERM].reshape(4, 128).T),
        g3=np.asarray(inputs['g3'], np.float32)[None, :],
        be3=np.asarray(inputs['be3'], np.float32)[None, :],
        linW=np.asarray(inputs['lin_W'], np.float32),
        linb=np.asarray(inputs['lin_b'], np.float32)[:, None],
        ident=np.eye(128, dtype=np.float32),
        msk=np.concatenate([np.ones((LAST, 1), np.float32),
                            np.zeros((128 - LAST, 1), np.float32)]),
        identb=np.eye(128, dtype=NP_BF16),
        indmat=np.broadcast_to((np.bincount(batch, minlength=G) > 0)
            .astype(np.float32)[None, :], (C, G)).copy(),
    )
    return tuple(T.tolist()), TT, per_core, shared


def _build(T_key, TT, repeat=1):
    T_list = list(T_key)
    nc = bacc.Bacc(None, target_bir_lowering=False, debug=False, num_devices=P,
                   num_swdge_queues=2)
    NE = TT * 128
    sbase = [0] * NT
    for b in range(1, NT):
        sbase[b] = sbase[b - 1] + T_list[b - 1]
    TMAXB = max(T_list)

    # ---- external inputs ----
    S_d = nc.dram_tensor("S", [128, NE], FP8, kind="ExternalInput")
    ST_d = nc.dram_tensor("ST", [128, NE], FP8, kind="ExternalInput")
    gidx_d = nc.dram_tensor("gidx", [128, NE // 16], I16, kind="ExternalInput")
    x0T_d = nc.dram_tensor("x0T", [FIN, NT * 128], BF16, kind="ExternalInput")
    pool_d = nc.dram_tensor("pool", [NT, 128, G], F32, kind="ExternalInput")
    rhs_t1_d = nc.dram_tensor("rhs_t1", [FIN, FIN], BF16, kind="ExternalInput")
    rhs_xe_d = nc.dram_tensor("rhs_xe", [FIN, FIN], BF16, kind="ExternalInput")
    rinv_d = {1: nc.dram_tensor("rinv1", [FIN, FIN], BF16, kind="ExternalInput"),
              2: nc.dram_tensor("rinv2", [FIN, FIN], BF16, kind="ExternalInput")}
    wtab2_d = nc.dram_tensor("wtab2", [FIN, FIN], BF16, kind="ExternalInput")
    w3_d = nc.dram_tensor("w3", [FIN, SS3], BF16, kind="ExternalInput")
    eb_t1_d = nc.dram_tensor("eb_t1", [1, FIN], BF16, kind="ExternalInput")
    eb_xe_d = nc.dram_tensor("eb_xe", [1, FIN], BF16, kind="ExternalInput")
    bnT_d = {ly: (nc.dram_tensor(f"g{ly}T", [128, 4], F32, kind="ExternalInput"),
                  nc.dram_tensor(f"be{ly}T", [128, 4], F32, kind="ExternalInput"))
             for ly in (1, 2)}
    g3_d = nc.dram_tensor("g3", [1, C], F32, kind="ExternalInput")
    be3_d = nc.dram_tensor("be3", [1, C], F32, kind="ExternalInput")
    linW_d = nc.dram_tensor("linW", [C, NCLS], F32, kind="ExternalInput")
    linb_d = nc.dram_tensor("linb", [NCLS, 1], F32, kind="ExternalInput")
    ident_d = nc.dram_tensor("ident", [128, 128], F32, kind="ExternalInput")
    identb_d = nc.dram_tensor("identb", [128, 128], BF16, kind="ExternalInput")
    indmat_d = nc.dram_tensor("indmat", [C, G], F32, kind="ExternalInput")
    msk_d = nc.dram_tensor("msk", [128, 1], F32, kind="ExternalInput")
    out_d = nc.dram_tensor("out", [G, NCLS], F32, kind="ExternalOutput")

    # ---- internal DRAM ----
    cc_in = {1: nc.dram_tensor("cc_in1", [NL, ROW12], BF16),
             2: nc.dram_tensor("cc_in2", [NL, ROW12], BF16),
             3: nc.dram_tensor("cc_in3", [NL, ROW3], BF16)}
    cc_out = {1: nc.dram_tensor("cc_out1", [N, ROW12], BF16, addr_space="Shared"),
              2: nc.dram_tensor("cc_out2", [N, ROW12], BF16, addr_space="Shared"),
              3: nc.dram_tensor("cc_out3", [N, ROW3], BF16, addr_space="Shared")}
    st_in = {1: nc.dram_tensor("st_in1", [128, 8], F32),
             2: nc.dram_tensor("st_in2", [128, 8], F32)}
    st_out = {1: nc.dram_tensor("st_out1", [P * 128, 8], F32, addr_space="Shared"),
              2: nc.dram_tensor("st_out2", [P * 128, 8], F32, addr_space="Shared")}
    ar3_in = nc.dram_tensor("ar3_in", [C + 2, G], F32)
    ar3_out = nc.dram_tensor("ar3_out", [(C + 2) * P, G], F32, addr_space="Shared")
    RG = [list(range(P))]

    with tile.TileContext(nc) as tc:
        with tc.tile_pool(name="cn", bufs=1) as cn, \
             tc.tile_pool(name="xb", bufs=1) as xb, \
             tc.tile_pool(name="gp", bufs=2) as gp, \
             tc.tile_pool(name="wp", bufs=2) as wp, \
             tc.tile_pool(name="sm", bufs=2) as sm, \
             tc.tile_pool(name="psA", bufs=2, space="PSUM") as psA, \
             tc.tile_pool(name="psB", bufs=1, space="PSUM") as psB, \
             tc.tile_pool(name="psU", bufs=2, space="PSUM") as psU:

            def cload(name, shape, dtype, dram, rearr=None, eng=None,
                      bufs=None, **kw):
                t = cn.tile(shape, dtype, tag=name, bufs=bufs)
                src = dram[:] if rearr is None else dram[:].rearrange(rearr, **kw)
                (eng or nc.gpsimd).dma_start(t[:], src)
                return t

            idx_sb = cload("idx", [128, NE // 16], I16, gidx_d,
                           eng=nc.scalar)
            pool_sb = cload("pool", [128, NT, G], F32, pool_d, "n p g -> p n g",
                            eng=nc.scalar)
            ident_sb = cload("ident", [128, 128], F32, ident_d, eng=nc.sync)
            identb_sb = cload("identb", [128, 128], BF16, identb_d,
                              eng=nc.sync)
            eb_t1_sb = cload("eb_t1", [1, FIN], BF16, eb_t1_d, eng=nc.sync)
            eb_xe_sb = cload("eb_xe", [1, FIN], BF16, eb_xe_d, eng=nc.sync)
            w3_sb = cload("w3", [128, 4, SS3], BF16, w3_d, "(k p) x -> p k x",
                          p=128, eng=nc.scalar)
            linW_sb = cload("linW", [C, NCLS], F32, linW_d, eng=nc.scalar)
            indmat_sb = cload("indmat", [C, G], F32, indmat_d, eng=nc.scalar)
            linb_sb = cload("linb", [NCLS, 1], F32, linb_d, eng=nc.scalar)
            bn3_sb = (cload("g3", [1, C], F32, g3_d, bufs=1),
                      cload("be3", [1, C], F32, be3_d, bufs=1))
            bnT_sb = {ly: (cload(f"g{ly}T", [128, 4], F32, bnT_d[ly][0], bufs=1),
                           cload(f"be{ly}T", [128, 4], F32, bnT_d[ly][1], bufs=1))
                      for ly in (1, 2)}
            # resident one-hot matrices (all 3 layers); spread loads
            S_res = cn.tile([128, TT, 128], FP8, tag="S_res")
            ST_res = cn.tile([128, TT, 128], FP8, tag="ST_res")
            half = (TT // 2) * 128

            ones_c = cn.tile([128, 1], BF16, tag="ones_c")
            nc.vector.memset(ones_c[:], 1.0)
            invN_c = cn.tile([128, 1], BF16, tag="invN_c")
            nc.vector.memset(invN_c[:], 1.0 / N)
            ones_cf = cn.tile([128, 1], F32, tag="ones_cf")
            nc.vector.memset(ones_cf[:], 1.0)
            invN_cf = cn.tile([128, 1], F32, tag="invN_cf")
            nc.vector.memset(invN_cf[:], 1.0 / N)
            eps_c = cn.tile([128, 1], F32, tag="eps_c")
            nc.vector.memset(eps_c[:], EPS_BN)
            zeros_c = cn.tile([128, 1], BF16, tag="zeros_c")
            nc.vector.memset(zeros_c[:], 0.0)
            ones_row = cn.tile([1, 128], BF16, tag="ones_row")
            nc.vector.memset(ones_row[:], 1.0)
            ebt1bc = cn.tile([128, FIN], BF16, tag="ebt1bc")
            nc.gpsimd.partition_broadcast(ebt1bc[:], eb_t1_sb[:])
            ebxerow = cn.tile([1, FIN], BF16, tag="ebxerow")
            nc.vector.tensor_copy(ebxerow[:], eb_xe_sb[:])

            # big node buffers
            hloc12 = xb.tile([128, NT, ROW12], BF16, tag="hloc12")  # table
            abuf = xb.tile([128, NT, FIN], BF16, tag="abuf")        # plain a
            ybuf = xb.tile([128, NT, FIN], BF16, tag="ybuf")        # y (std-ilv)
            hloc3t = xb.tile([128, NT, SS3], BF16, tag="hloc3")
            msk_c = cload("msk", [128, 1], F32, msk_d, eng=nc.sync)

            def nvalid(n):
                return 128 if n < NT - 1 else LAST

            def ccag(ly):
                nc.gpsimd.collective_compute(
                     "AllGather", AL.bypass, replica_groups=RG,
                    ins=[cc_in[ly][:]], outs=[cc_out[ly][:]])

            # ---------- L1 table phase: t1 = x @ RHS_t1 + eb ----------
            def table1_phase(lhsT_sb, rhs_sb):
                for n in range(NT):
                    # alternate PSUM pools: psU's banks are idle pre-edge,
                    # doubling the rotation depth for the matmul->add chain
                    if n % 2 == 0:
                        p5 = psA.tile([128, FIN], F32, tag="mm5")
                    else:
                        p5 = psU.tile([128, FIN], F32, tag="U")
                    for k in range(4):
                        nc.tensor.matmul(p5[:],
                                         lhsT_sb[:, k, 128 * n:128 * (n + 1)],
                                         rhs_sb[:, k, :],
                                         start=(k == 0), stop=(k == 3))
                    nc.vector.tensor_tensor(out=hloc12[:, n], in0=p5[:],
                                            in1=ebt1bc[:], op=AL.add)
                    v = nvalid(n)
                    nc.sync.dma_start(cc_in[1][128 * n:128 * n + v, :],
                                      hloc12[0:v, n, :])

            # xe = x_enc plain (residual base); fills PE gaps in edge1
            def xe_phase(lhsT_sb, rhs_sb):
                with tc.high_priority(offset=-500000):
                    for n in range(NT):
                        pxe = psA.tile([128, FIN], F32, tag="mm5", name="pxe")
                        nc.tensor.matmul(pxe[:], ones_row[:], ebxerow[:],
                                         start=True, stop=False,
                                         skip_group_check=True)
                        for k in range(4):
                            nc.tensor.matmul(
                                pxe[:], lhsT_sb[:, k, 128 * n:128 * (n + 1)],
                                rhs_sb[:, k, :], start=False, stop=(k == 3),
                                skip_group_check=True)
                        nc.scalar.copy(abuf[:, n], pxe[:])

            # ---------- edge aggregation phase ----------
            ORDER = [NT - 1] + list(range(NT - 1))

            def edge_phase(ly, nh, fh, ybuf_l, pstA, pstB, hloc, rinv_sb):
                cph = fh // nh
                rowv = ROW12 if ly < 3 else ROW3
                # L1/L2: sD at ilv 496:504, sS at 504:512 (inside payload)
                # L3: payload h3 0:64, sS at 64, sD at 65 (sS gathered too)
                sd_of = fh - 2 * nh if ly < 3 else fh + nh
                ss_of = fh - nh if ly < 3 else fh
                gt = "g" if ly < 3 else "g3"
                order = ORDER

                def emit_tailA(st):
                    # division U/Z -> ya (or straight into ybuf_l for nh==1)
                    st[4] = _edge_tailA(ly, nh, fh, cph, ybuf_l, st[1], st[2],
                                        st[3])
                    st[5] = True

                def emit_tailB(st):
                    _edge_tailB(ly, nh, fh, cph, ybuf_l, pstA, pstB, rinv_sb,
                                st[0], st[1], st[4])
                # self-loop prep for ALL blocks, hoisted so it runs on
                # DVE/Act UNDER the collective (Pool is busy with it)
                ws_all = wp.tile([128, NT, 8], BF16, tag="ws_all", bufs=2)
                slw_all = wp.tile([128, NT, FIN], BF16, tag="slw_all", bufs=1)
                for b in ORDER:
                    lgs = wp.tile([128, 8], F32, tag="lgs")
                    nc.vector.tensor_tensor(
                        out=lgs[:, 0:nh], in0=hloc[:, b, ss_of:ss_of + nh],
                        in1=hloc[:, b, sd_of:sd_of + nh], op=AL.add)
                    nc.vector.scalar_tensor_tensor(
                        out=lgs[:, 0:nh], in0=lgs[:, 0:nh], scalar=0.2,
                        in1=lgs[:, 0:nh], op0=AL.mult, op1=AL.max)
                    nc.scalar.activation(ws_all[:, b, 0:nh], lgs[:, 0:nh],
                                         ACTF.Exp)
                    if b == NT - 1:
                        nc.vector.tensor_scalar_mul(
                            ws_all[:, b, 0:nh], ws_all[:, b, 0:nh], msk_c[:])
                    if nh == 8:
                        nc.vector.tensor_tensor(
                            out=slw_all[:, b, 0:fh].rearrange(
                                "p (c h) -> p c h", h=nh),
                            in0=hloc[:, b, 0:fh].rearrange(
                                "p (c h) -> p c h", h=nh),
                            in1=ws_all[:, b, :].unsqueeze(1).broadcast_to(
                                [128, cph, nh]),
                            op=AL.mult)
                    else:
                        nc.vector.tensor_copy(ws_all[:, b, 1:2],
                                              ws_all[:, b, 0:1])
                        nc.vector.tensor_tensor(
                            out=slw_all[:, b, 0:fh].rearrange(
                                "p (q r) -> p q r", r=2),
                            in0=hloc[:, b, 0:fh].rearrange(
                                "p (q r) -> p q r", r=2),
                            in1=ws_all[:, b, 0:2].unsqueeze(1).broadcast_to(
                                [128, fh // 2, 2]),
                            op=AL.mult)
                pend = None
                for pos, b in enumerate(order):
                    T = T_list[b]
                    s0 = sbase[b]
                    w_t = wp.tile([128, TMAXB, 8], BF16, tag="w_t")
                    pU = psU.tile([128, FIN], F32, tag="U")
                    pZ = psB.tile([128, 8], F32, tag="Z")
                    nc.tensor.matmul(pU[:, 0:fh], identb_sb[:],
                                     slw_all[:, b, 0:fh],
                                     start=True, stop=False,
                                     skip_group_check=True)
                    nc.tensor.matmul(pZ[:, 0:nh], identb_sb[:],
                                     ws_all[:, b, 0:nh],
                                     start=True, stop=False,
                                     skip_group_check=True)
                    if b == order[-1]:
                        sched = [8, 4, 2, 2]
                    elif b == order[0]:
                        sched = [2, 2, 4, 8]
                    else:
                        sched = [8, 8]
                    c0 = 0
                    for ic, chs in enumerate(sched):
                        nsl = min(chs, T - c0)
                        if nsl <= 0:
                            break
                        sg = s0 + c0
                        g = gp.tile([128, CH, rowv], BF16, tag=gt, bufs=5)
                        nc.gpsimd.dma_gather(
                            g[:, 0:nsl, 0:rowv], cc_out[ly][:],
                            idx_sb[:, 8 * sg:8 * (sg + nsl)],
                            num_idxs=nsl * 128, num_idxs_reg=nsl * 128,
                            elem_size=rowv, queue_num=0)
                        psd = psB.tile([128, CH * 8], F32, tag="sd", bufs=2)
                        for t in range(nsl):
                            nc.tensor.matmul(
                                psd[:, t * nh:(t + 1) * nh],
                                ST_res[:, sg + t, :],
                                hloc[:, b, sd_of:sd_of + nh],
                                start=True, stop=True)
                        lg = wp.tile([128, CH * 8], F32, tag="lg", bufs=4)
                        nc.vector.tensor_tensor(
                            out=lg[:, 0:nsl * nh],
                            in0=g[:, 0:nsl, ss_of:ss_of + nh],
                            in1=psd[:, 0:nsl * nh], op=AL.add)
                        nc.vector.scalar_tensor_tensor(
                            out=lg[:, 0:nsl * nh], in0=lg[:, 0:nsl * nh],
                            scalar=0.2, in1=lg[:, 0:nsl * nh],
                            op0=AL.mult, op1=AL.max)
                        nc.scalar.activation(
                            w_t[:, c0:c0 + nsl, 0:nh], lg[:, 0:nsl * nh],
                            ACTF.Exp)
                        if nh == 1:
                            nc.scalar.activation(
                                w_t[:, c0:c0 + nsl, 1:2], lg[:, 0:nsl],
                                ACTF.Exp)
                        if nh == 8:
                            nc.vector.tensor_tensor(
                                out=g[:, 0:nsl, 0:fh].rearrange(
                                    "p t (c h) -> p t c h", h=nh),
                                in0=g[:, 0:nsl, 0:fh].rearrange(
                                    "p t (c h) -> p t c h", h=nh),
                                in1=w_t[:, c0:c0 + nsl, :].unsqueeze(2)
                                    .broadcast_to([128, nsl, cph, nh]),
                                op=AL.mult)
                        else:
                            # pair view: last dim [2] packed -> DVE 2x
                            nc.vector.tensor_tensor(
                                out=g[:, 0:nsl, 0:fh].rearrange(
                                    "p t (q r) -> p t q r", r=2),
                                in0=g[:, 0:nsl, 0:fh].rearrange(
                                    "p t (q r) -> p t q r", r=2),
                                in1=w_t[:, c0:c0 + nsl, 0:2].unsqueeze(2)
                                    .broadcast_to([128, nsl, cph // 2, 2]),
                                op=AL.mult)
                        for t in range(nsl):
                            nc.tensor.matmul(
                                pU[:, 0:fh], S_res[:, sg + t, :], g[:, t, 0:fh],
                                start=False, stop=(c0 + t == T - 1),
                                skip_group_check=True)
                            nc.tensor.matmul(
                                pZ[:, 0:nh], S_res[:, sg + t, :],
                                w_t[:, c0 + t, 0:nh],
                                start=False, stop=(c0 + t == T - 1),
                                skip_group_check=True)
                        c0 += nsl
                    rz = sm.tile([128, 8], F32, tag="rz")
                    nc.vector.tensor_scalar_add(rz[:, 0:nh], pZ[:, 0:nh], EPS_Z)
                    nc.vector.reciprocal(rz[:, 0:nh], rz[:, 0:nh])
                    # software pipelining: the previous block's division was
                    # emitted between this block's chunks (ic==1 hook); the
                    # rest of its tail goes here.
                    if pend is not None:
                        if not pend[5]:
                            emit_tailA(pend)
                        emit_tailB(pend)
                    pend = [pos, b, pU, rz, None, False]
                emit_tailA(pend)
                emit_tailB(pend)

            def _edge_tailA(ly, nh, fh, cph, ybuf_l, b, pU, rz):
                    if nh == 8:
                        # yagg = U/Z (table basis)
                        ya = sm.tile([128, FIN], BF16, tag="ya")
                        nc.vector.tensor_tensor(
                            out=ya[:].rearrange("p (c h) -> p c h", h=nh),
                            in0=pU[:].rearrange("p (c h) -> p c h", h=nh),
                            in1=rz[:, 0:nh].unsqueeze(1).broadcast_to(
                                [128, cph, nh]),
                            op=AL.mult)
                        return ya
                    nc.vector.tensor_tensor(
                        out=ybuf_l[:, b, 0:fh], in0=pU[:, 0:fh],
                        in1=rz[:, 0:nh].unsqueeze(2).broadcast_to(
                            [128, nh, cph]),
                        op=AL.mult)
                    return None

            def _edge_tailB(ly, nh, fh, cph, ybuf_l, pstA, pstB, rinv_sb,
                            pos, b, ya):
                    if nh == 8:
                        # transpose, recover std basis: y = yagg @ Rinv
                        psT = psB.tile([128, 4, 128], BF16, tag="sd", bufs=2,
                                       name="psT")
                        for k in range(4):
                            nc.tensor.transpose(
                                psT[:, k, :], ya[:, 128 * k:128 * (k + 1)],
                                identb_sb[:])
                        yaT = sm.tile([128, 4, 128], BF16, tag="yaT")
                        nc.scalar.copy(yaT[:], psT[:])
                        pW = psA.tile([128, FIN], F32, tag="mm5", name="pW")
                        for k in range(4):
                            nc.tensor.matmul(pW[:], yaT[:, k, :],
                                             rinv_sb[:, k, :],
                                             start=(k == 0), stop=(k == 3))
                        nc.scalar.copy(ybuf_l[:, b, :], pW[:])
                        y2 = sm.tile([128, FIN], BF16, tag="y2")
                        nc.scalar.activation(y2[:], pW[:], ACTF.Square)
                        if pos == 0:
                            for kk in range(8):
                                nc.tensor.matmul(
                                    pstA[:, kk:kk + 1], identb_sb[:],
                                    zeros_c[:], start=True, stop=False,
                                    skip_group_check=True)
                        for k in range(4):
                            nc.tensor.matmul(
                                pstA[:, k:k + 1],
                                ybuf_l[:, b, 128 * k:128 * (k + 1)], invN_c[:],
                                start=False, stop=(pos == NT - 1),
                                skip_group_check=True)
                            nc.tensor.matmul(
                                pstA[:, 4 + k:5 + k],
                                y2[:, 128 * k:128 * (k + 1)], invN_c[:],
                                start=False, stop=(pos == NT - 1),
                                skip_group_check=True)
                    else:
                        y2 = sm.tile([128, FIN], BF16, tag="y2")
                        nc.scalar.activation(y2[:, 0:fh], ybuf_l[:, b, 0:fh],
                                             ACTF.Square)
                        nc.tensor.matmul(pstA[:, 0:fh], invN_cf[:],
                                         ybuf_l[:, b, 0:fh], start=(pos == 0),
                                         stop=(pos == NT - 1),
                                         skip_group_check=True)
                        nc.tensor.matmul(pstB[:, 0:fh], invN_c[:], y2[:, 0:fh],
                                         start=(pos == 0), stop=(pos == NT - 1),
                                         skip_group_check=True)
                        nc.tensor.matmul(pxg_t[:], ybuf_l[:, b, :],
                                         pool_sb[:, b, :], start=(pos == 0),
                                         stop=(pos == NT - 1),
                                         skip_group_check=True)

            # ---------- BN + ELU + residual + next-layer table ----------
            def bn_chain(ly, pst1, wtab_sb):
                gT_sb, beT_sb = bnT_sb[ly]
                statT = sm.tile([128, 8], F32, tag="statT", bufs=1)
                nc.vector.tensor_copy(statT[:], pst1[:, 0:8])
                nc.sync.dma_start(st_in[ly][:], statT[:])
                nc.gpsimd.collective_compute(
                     "AllGather", AL.bypass, replica_groups=RG,
                    ins=[st_in[ly][:]], outs=[st_out[ly][:]])
                st8 = sm.tile([128, P, 8], F32, tag="st8", bufs=1)
                nc.sync.dma_start(
                    st8[:], st_out[ly][:].rearrange("(r p) c -> p r c", p=128))
                ss = sm.tile([128, 8], F32, tag="sstat", bufs=1)
                nc.vector.tensor_reduce(
                    out=ss[:], in_=st8[:].rearrange("p r c -> p c r"),
                    axis=AX.X, op=AL.add)
                mu = ss[:, 0:4]
                isd = sm.tile([128, 4], F32, tag="isdT", bufs=1)
                nc.vector.tensor_tensor(out=isd[:], in0=mu[:], in1=mu[:],
                                        op=AL.mult)
                nc.vector.tensor_tensor(out=isd[:], in0=ss[:, 4:8],
                                        in1=isd[:], op=AL.subtract)
                nc.scalar.activation(isd[:], isd[:], ACTF.Ln, bias=eps_c[:])
                nc.scalar.activation(isd[:], isd[:], ACTF.Exp, scale=-0.5)
                scfT = sm.tile([128, 4], F32, tag="scfT", bufs=1)
                shfT = sm.tile([128, 4], F32, tag="shfT", bufs=1)
                nc.vector.tensor_tensor(out=scfT[:], in0=gT_sb[:],
                                        in1=isd[:], op=AL.mult)
                nc.vector.tensor_tensor(out=shfT[:], in0=scfT[:],
                                        in1=mu[:], op=AL.mult)
                nc.vector.tensor_tensor(out=shfT[:], in0=beT_sb[:],
                                        in1=shfT[:], op=AL.subtract)
                scfTb = sm.tile([128, 8], BF16, tag="scfTb", bufs=1)
                nc.vector.tensor_copy(scfTb[:, 0:4], scfT[:])
                nc.vector.tensor_copy(scfTb[:, 4:8], shfT[:])
                psc2 = psB.tile([1, 4, 128], BF16, tag="sd", bufs=2, name="psc2")
                psc3 = psB.tile([1, 4, 128], BF16, tag="sd", bufs=2, name="psc3")
                for k in range(4):
                    nc.tensor.transpose(psc2[:, k, :], scfTb[:, k:k + 1],
                                        identb_sb[:])
                    nc.tensor.transpose(psc3[:, k, :], scfTb[:, 4 + k:5 + k],
                                        identb_sb[:])
                row4 = sm.tile([1, 8, 128], BF16, tag="row4", bufs=1)
                nc.vector.tensor_copy(row4[:, 0:4, :], psc2[:])
                nc.vector.tensor_copy(row4[:, 4:8, :], psc3[:])
                # broadcast rows -> [128, 512] via rank-1 matmul (PE is free
                # here; Pool partition_broadcast would serialize the prologue)
                scT = sm.tile([128, FIN], F32, tag="scT", bufs=1)
                shT = sm.tile([128, FIN], F32, tag="shT", bufs=1)
                pbc = psA.tile([128, FIN], F32, tag="mm5", name="pbc")
                nc.tensor.matmul(pbc[:], ones_row[:], row4[:, 0:4, :],
                                 start=True, stop=True)
                nc.scalar.copy(scT[:], pbc[:])
                pbc2 = psA.tile([128, FIN], F32, tag="mm5", name="pbc2")
                nc.tensor.matmul(pbc2[:], ones_row[:], row4[:, 4:8, :],
                                 start=True, stop=True)
                nc.scalar.copy(shT[:], pbc2[:])
                # per tile: a' = elu(scT*y + shT) + a; table' = a' @ Wtab
                # software-pipelined: tile n's table tail is emitted after
                # tile n+1's elu head so the Act/DVE queues don't stall on
                # the PSUM table copy.
                def bn_tail(n):
                    psT = psB.tile([128, 4, 128], BF16, tag="sd", bufs=2,
                                   name="psTa")
                    for k in range(4):
                        nc.tensor.transpose(
                            psT[:, k, :], abuf[:, n, 128 * k:128 * (k + 1)],
                            identb_sb[:])
                    aT = sm.tile([128, 4, 128], BF16, tag="yaT", name="aT")
                    nc.vector.tensor_copy(aT[:], psT[:])
                    vv = nvalid(n)
                    if ly == 1:
                        if n % 2 == 0:
                            pP = psA.tile([128, FIN], F32, tag="mm5", name="pP")
                        else:
                            pP = psU.tile([128, FIN], F32, tag="U", name="pP")
                        for k in range(4):
                            nc.tensor.matmul(pP[:], aT[:, k, :],
                                             wtab_sb[:, k, :],
                                             start=(k == 0), stop=(k == 3))
                        nc.scalar.copy(hloc12[:, n], pP[:])
                        nc.sync.dma_start(cc_in[2][128 * n:128 * n + vv, :],
                                          hloc12[0:vv, n, :])
                    else:
                        pP = psB.tile([128, SS3], F32, tag="Z", name="pP3")
                        for k in range(4):
                            nc.tensor.matmul(pP[:, 0:SS3], aT[:, k, :],
                                             wtab_sb[:, k, 0:SS3],
                                             start=(k == 0), stop=(k == 3))
                        nc.scalar.copy(hloc3t[:, n, 0:SS3], pP[:, 0:SS3])
                        nc.sync.dma_start(cc_in[3][128 * n:128 * n + vv, 0:SS3],
                                          hloc3t[0:vv, n, 0:SS3])

                for n in range(NT):
                    v = sm.tile([128, FIN], BF16, tag="cht", bufs=3, name="v")
                    nc.gpsimd.tensor_tensor(out=v[:], in0=ybuf[:, n],
                                            in1=scT[:], op=AL.mult)
                    nc.gpsimd.tensor_tensor(out=v[:], in0=v[:],
                                            in1=shT[:], op=AL.add)
                    # elu(v) = max(v,0) + min(exp(v)-1, 0): exp overflow on
                    # the positive side saturates to inf and min() discards it
                    m = sm.tile([128, FIN], BF16, tag="che", bufs=3, name="m")
                    nc.scalar.activation(m[:], v[:], ACTF.Exp)
                    nc.vector.tensor_scalar(m[:], m[:], -1.0, 0.0,
                                            AL.add, AL.min)
                    xm = sm.tile([128, FIN], BF16, tag="chx", bufs=3, name="xm")
                    nc.vector.tensor_tensor(out=xm[:], in0=m[:],
                                            in1=abuf[:, n], op=AL.add)
                    nc.vector.tensor_scalar_max(v[:], v[:], 0.0)
                    nc.vector.tensor_tensor(out=abuf[:, n], in0=v[:],
                                            in1=xm[:], op=AL.add)
                    if n > 0:
                        bn_tail(n - 1)
                bn_tail(NT - 1)

            # =========== emit program ===========
            for _rep in range(repeat):
              x0T_sb = xb.tile([128, 4, NT * 128], BF16, tag="lhsT",
                               name="x0T_sb")
              nc.sync.dma_start(x0T_sb[:],
                                x0T_d[:].rearrange("(k p) x -> p k x", p=128))
              rhs_t1_sb = cn.tile([128, 4, FIN], BF16, tag="rhs_t1")
              nc.scalar.dma_start(rhs_t1_sb[:],
                                  rhs_t1_d[:].rearrange("(k p) x -> p k x", p=128))
              rhs_xe_sb = cn.tile([128, 4, FIN], BF16, tag="rhs_xe")
              nc.scalar.dma_start(rhs_xe_sb[:],
                                  rhs_xe_d[:].rearrange("(k p) x -> p k x", p=128))
              rinv1_sb = cn.tile([128, 4, FIN], BF16, tag="rinv1")
              nc.scalar.dma_start(rinv1_sb[:],
                                  rinv_d[1][:].rearrange("(k p) x -> p k x", p=128))

              # L1
              table1_phase(x0T_sb, rhs_t1_sb)
              # one-hot loads ride behind table1 (needed only at edge1)
              nc.sync.dma_start(S_res[:, 0:TT // 2, :], S_d[:, 0:half])
              nc.scalar.dma_start(S_res[:, TT // 2:TT, :], S_d[:, half:NE])
              nc.sync.dma_start(ST_res[:, 0:TT // 2, :], ST_d[:, 0:half])
              nc.scalar.dma_start(ST_res[:, TT // 2:TT, :], ST_d[:, half:NE])
              ccag(1)
              pstA1 = psB.tile([128, 8], F32, tag="pstT", bufs=1, name="pstA1")
              xe_phase(x0T_sb, rhs_xe_sb)
              edge_phase(1, H, FIN, ybuf, pstA1, None, hloc12, rinv1_sb)
              rinv2_sb = cn.tile([128, 4, FIN], BF16, tag="rhs_xe",
                                 name="rinv2_sb")
              nc.scalar.dma_start(rinv2_sb[:],
                                  rinv_d[2][:].rearrange("(k p) x -> p k x", p=128))
              wtab2_sb = cn.tile([128, 4, FIN], BF16, tag="rhs_t1",
                                 name="wtab2_sb")
              nc.sync.dma_start(wtab2_sb[:],
                                wtab2_d[:].rearrange("(k p) x -> p k x", p=128))
              bn_chain(1, pstA1, wtab2_sb)

              # L2
              ccag(2)
              pstA2 = psB.tile([128, 8], F32, tag="pstT", bufs=1, name="pstA2")
              edge_phase(2, H, FIN, ybuf, pstA2, None, hloc12, rinv2_sb)
              bn_chain(2, pstA2, w3_sb)

              # L3 (y3 reuses x0T's slot: x0T is dead after xe_phase)
              y3 = xb.tile([128, NT, C], F32, tag="lhsT", name="y3")
              ccag(3)
              pstA3 = psA.tile([1, FIN], F32, tag="mm5", name="pstA3")
              pstB3 = psA.tile([1, FIN], F32, tag="mm5", name="pstB3")
              pxg_t = psB.tile([C, G], F32, tag="pstT", bufs=1, name="pxg")
              edge_phase(3, 1, C, y3, pstA3, pstB3, hloc3t, None)

              # L3 stats + pooled sums, one AllGather for both
              stat3 = sm.tile([1, 2 * C], F32, tag="stat", name="stat3", bufs=1)
              nc.scalar.copy(stat3[:, 0:C], pstA3[:, 0:C])
              nc.scalar.copy(stat3[:, C:2 * C], pstB3[:, 0:C])
              xg = sm.tile([C, G], F32, tag="xg")
              nc.scalar.copy(xg[:], pxg_t[:])
              nc.sync.dma_start(ar3_in[0:C, :], xg[:])
              nc.sync.dma_start(ar3_in[C:C + 1, :], stat3[:, 0:C])
              nc.sync.dma_start(ar3_in[C + 1:C + 2, :], stat3[:, C:2 * C])
              nc.gpsimd.collective_compute(
                   "AllGather", AL.bypass, replica_groups=RG,
                  ins=[ar3_in[:]], outs=[ar3_out[:]])
              pooled8 = sm.tile([C, P, G], F32, tag="pooled8", bufs=1)
              nc.sync.dma_start(
                  pooled8[:, :, :],
                  ar3_out[:].rearrange("(r i) g -> i r g", r=P)[0:C])
              yg2 = sm.tile([C, G], F32, tag="xg2")
              nc.vector.tensor_reduce(
                  out=yg2[:, :],
                  in_=pooled8[:, :, :].rearrange("i r g -> i g r"),
                  axis=AX.X, op=AL.add)
              st8b = sm.tile([P, 2 * C], F32, tag="st8", bufs=1, name="st8b")
              nc.scalar.dma_start(
                  st8b[:, :],
                  ar3_out[:].rearrange("(r i) g -> r (i g)", r=P)
                  [:, C * G:C * G + 2 * C])
              pm3 = psA.tile([1, FIN], F32, tag="mm5", name="pm3")
              nc.tensor.matmul(pm3[:, 0:2 * C], ones_cf[0:P, :],
                               st8b[:, :], start=True, stop=True)
              st3 = sm.tile([1, 2 * C], F32, tag="stat2", name="st3", bufs=1)
              nc.scalar.copy(st3[:, 0:2 * C], pm3[:, 0:2 * C])
              mu3 = st3[:, 0:C]
              ex23 = st3[:, C:2 * C]
              var3 = sm.tile([1, C], F32, tag="var", name="var3", bufs=1)
              nc.vector.tensor_tensor(out=var3[:, 0:C], in0=mu3, in1=mu3,
                                      op=AL.mult)
              nc.vector.tensor_tensor(out=var3[:, 0:C], in0=ex23,
                                      in1=var3[:, 0:C], op=AL.subtract)
              sd3 = sm.tile([1, C], F32, tag="sdv", name="sd3", bufs=1)
              nc.vector.tensor_scalar_add(var3[:, 0:C], var3[:, 0:C], EPS_BN)
              nc.scalar.activation(sd3[:, 0:C], var3[:, 0:C], ACTF.Ln)
              nc.scalar.activation(sd3[:, 0:C], sd3[:, 0:C], ACTF.Exp,
                                   scale=-0.5)
              g3_sb, be3_sb = bn3_sb
              scf3 = sm.tile([1, C], F32, tag="scf", name="scf3", bufs=1)
              nc.vector.tensor_tensor(out=scf3[:, 0:C], in0=g3_sb[:],
                                      in1=sd3[:, 0:C], op=AL.mult)
              shf3 = sm.tile([1, C], F32, tag="shf", name="shf3", bufs=1)
              nc.vector.tensor_tensor(out=shf3[:, 0:C], in0=scf3[:, 0:C],
                                      in1=mu3, op=AL.mult)
              nc.vector.tensor_tensor(out=shf3[:, 0:C], in0=be3_sb[:],
                                      in1=shf3[:, 0:C], op=AL.subtract)
              psc = psB.tile([C, 1], F32, tag="Z", name="psc")
              nc.tensor.transpose(psc[:], scf3[:, 0:C], ident_sb[0:1, 0:1])
              scol = sm.tile([C, 1], F32, tag="scol", name="scol")
              nc.scalar.copy(scol[:], psc[:])
              psh = psB.tile([C, 1], F32, tag="Z", name="psh")
              nc.tensor.transpose(psh[:], shf3[:, 0:C], ident_sb[0:1, 0:1])
              shcol = sm.tile([C, 1], F32, tag="shcol", name="shcol")
              nc.scalar.copy(shcol[:], psh[:])
              lws = sm.tile([C, NCLS], F32, tag="lws", name="lws")
              nc.vector.tensor_scalar_mul(lws[:], linW_sb[:], scol[:])
              pb2 = psB.tile([NCLS, 1], F32, tag="Z", name="pb2")
              nc.tensor.matmul(pb2[:], linW_sb[:], shcol[:], start=True,
                               stop=True)
              bsum = sm.tile([NCLS, 1], F32, tag="bsum", name="bsum")
              nc.vector.tensor_tensor(out=bsum[:], in0=pb2[:],
                                      in1=linb_sb[:], op=AL.add)
              pot = psB.tile([NCLS, G], F32, tag="sd", bufs=2, name="pot")
              nc.tensor.matmul(pot[:], lws[:], yg2[:], start=True,
                               stop=True)
              outT = sm.tile([NCLS, G], F32, tag="outT")
              nc.scalar.activation(outT[:], pot[:], ACTF.Identity,
                                   bias=bsum[:])
              pfin = psB.tile([G, NCLS], F32, tag="sd", bufs=2, name="pfin")
              nc.tensor.transpose(pfin[:], outT[:], ident_sb[0:NCLS, 0:NCLS])
              fin = sm.tile([G, NCLS], F32, tag="fin_sb")
              nc.vector.tensor_copy(fin[:], pfin[:])
              nc.sync.dma_start(out_d[:], fin[:])

        sched_state, snap = tc.schedule_and_allocate()
        nc._sched_state = sched_state
        nc._pred_ns = snap.time

    nc.finalize()
    return nc


_CACHE = {}


def _get_nc(T_key, TT, repeat=1):
    key = (T_key, repeat)
    if key not in _CACHE:
        _CACHE[key] = _build(T_key, TT, repeat)
    return _CACHE[key]


def make_in_maps(per_core, shared):
    return [dict(S=pc['S'], ST=pc['ST'], gidx=pc['gidx'],
                 x0T=pc['x0T'], pool=pc['pool'], **shared)
            for pc in per_core]


def kernel(**inputs):
    T_key, TT, per_core, shared = _prep(inputs)
    nc = _get_nc(T_key, TT)
    in_maps = make_in_maps(per_core, shared)
    res = run_bass_kernel_spmd(nc, in_maps, core_ids=list(range(P)))
    return np.asarray(res.results[0]['out'], np.float32)
